# revision 8
# baseline (speedup 1.0000x reference)
"""Trainium2 Bass kernel for nn_CheapChannelV1 (dense_cnn).

Strategy (per core, pure data-parallel over batch — one sample per core):
  - The three channel-shuffle + 1x1-conv stages are linear, so they fold on the
    host into ONE 128x128 matrix M and bias b_tot:  res3 = M @ s + b_tot, where
    s = [s0;s1;s2;s3] are the four depthwise-conv branch outputs.
  - All matmul operands are bf16 (fp32 PSUM accumulation): fp32 matmuls run at
    4 cycles/column on the PE vs 1 for bf16.  x is cast to bf16 on the host,
    which also halves the HBM read traffic.
  - Level-0 depthwise conv (full res) folds INTO the matmul: 9 tap matmuls
    (K=32) reading shifted views of a host-prepadded x0 strip (channels 0-31
    replicated across the four 32-partition groups, one group per row-block).
  - Levels 1-3: hierarchical 2x2 max-pool on DVE in 8-row half-bands
    (vertical-first so half the ops hit the 16-bit 2x mode); 3x3 depthwise
    conv over 16-row bands (level 1 on DVE, levels 2+3 on GPSIMD);
    nearest-upsample folds into broadcast (step-0) rhs APs of the matmuls.
  - Pooling runs one half-band ahead of compute so the conv's +1-row halo
    dependency never serializes the band pipeline.  Block-boundary halo rows
    are seeded from a tiny host-computed init tensor.
  - 12 accumulating K=32 matmuls per 512-px chunk, spread across the four PE
    row groups via tile_position for 4x concurrency.
  - Epilogue: exact Gelu on ACT (bias folded in, bf16 out), multiply-by-x on
    DVE in bf16 (2x mode), store via SWDGE cast-DMA (bf16 SBUF -> fp32 HBM).
"""

import numpy as np
import ml_dtypes

BF16 = ml_dtypes.bfloat16

H = W = 256
CH = 128
NC_ = 4       # compute bands ("cbands") of 16 rows per row-block
CB = 16       # rows per cband
HB = 8        # half-band rows (pooling granularity)

STORE_CAST = True     # bf16 mul output + SWDGE cast store (else fp32 + sync)
CONV23_GPSIMD = False  # GPSIMD lacks the scalar_tensor_tensor opcode


def _shuf_cols(A, groups=8):
    # Returns A' with A' @ s == A @ channel_shuffle(s)
    Cin = A.shape[1]
    idx = np.arange(Cin)
    perm = (idx % groups) * (Cin // groups) + idx // groups
    Ap = np.zeros_like(A)
    Ap[:, perm] = A
    return Ap


def fold_weights(w_dw, b_dw, w_f1, b_f1, w_f2, b_f2, w_f3, b_f3):
    f8 = np.float64
    A1 = _shuf_cols(w_f1.astype(f8))
    A2 = _shuf_cols(w_f2.astype(f8))
    A3 = _shuf_cols(w_f3.astype(f8))
    A2a, A2b = A2[:, :64], A2[:, 64:]
    A3a, A3b = A3[:, :96], A3[:, 96:]
    M = np.zeros((128, 128), f8)
    M[:, 0:64] = A3a @ A2a @ A1
    M[:, 64:96] = A3a @ A2b
    M[:, 96:128] = A3b
    b_tot = A3a @ (A2a @ b_f1.astype(f8) + b_f2.astype(f8)) + b_f3.astype(f8)
    for g in range(4):
        b_tot = b_tot + M[:, 32 * g:32 * g + 32] @ b_dw[g].astype(f8)

    # W_all[p, t, o]: lhsT matrices, identical content per 32-partition group.
    W_all = np.zeros((128, 12, 128), np.float32)
    M0T = M[:, 0:32].T          # [32(c), 128(o)]
    w0 = w_dw[0].reshape(32, 9).astype(f8)
    for gp in range(4):
        rows = slice(32 * gp, 32 * gp + 32)
        for j in range(9):
            W_all[rows, j, :] = (M0T * w0[:, j:j + 1]).astype(np.float32)
        W_all[rows, 9, :] = M[:, 32:64].T.astype(np.float32)
        W_all[rows, 10, :] = M[:, 64:96].T.astype(np.float32)
        W_all[rows, 11, :] = M[:, 96:128].T.astype(np.float32)

    wdwp = np.zeros((128, 3, 9), np.float32)
    for g in (1, 2, 3):
        wdwp[:, g - 1, :] = np.tile(w_dw[g].reshape(32, 9), (4, 1)).astype(np.float32)

    return (np.ascontiguousarray(W_all.astype(BF16)),
            b_tot.astype(np.float32).reshape(128, 1),
            wdwp)


def _pool2d(a, k):
    # a: [C, R, W] -> max-pooled [C, R//k, W//k]
    C, R, Ww = a.shape
    return a.reshape(C, R // k, k, Ww // k, k).max(axis=(2, 4))


def prep_sample(x):
    """Host-side layout/dtype prep for one sample x [128, 256, 256] fp32."""
    xb = x.astype(BF16)

    # x0 strip: channels 0-31 replicated to the 4 row-block partition groups,
    # pre-padded; cband c rows are image rows 16c-1 .. 16c+17 (block-local),
    # cols padded by 1 on each side.
    xp = np.zeros((32, H + 2, W + 2), BF16)
    xp[:, 1:H + 1, 1:W + 1] = xb[:32]
    rows = (np.arange(4)[:, None, None] * 64
            + np.arange(NC_)[None, :, None] * CB
            + np.arange(CB + 2)[None, None, :])       # [4, 4, 18] (+1 pad -1)
    x0 = xp[:, rows.reshape(-1), :]                    # [32, 288, 258]
    x0 = np.ascontiguousarray(
        x0.reshape(32, 4, NC_ * (CB + 2), W + 2).transpose(1, 0, 2, 3)
        .reshape(128, NC_ * (CB + 2), W + 2))

    # Pool-strip pad inits: zeros + block-boundary halo rows.
    p1i = np.zeros((128, 34, 130), BF16)
    p2i = np.zeros((128, 18, 66), BF16)
    p3i = np.zeros((128, 10, 34), BF16)
    for r in range(4):
        g = 32 * r
        if r > 0:   # top halos: last pooled row of block r-1
            p1i[g:g + 32, 0, 1:129] = _pool2d(xb[32:64, 64 * r - 2:64 * r], 2)[:, 0]
            p2i[g:g + 32, 0, 1:65] = _pool2d(xb[64:96, 64 * r - 4:64 * r], 4)[:, 0]
            p3i[g:g + 32, 0, 1:33] = _pool2d(xb[96:128, 64 * r - 8:64 * r], 8)[:, 0]
        if r < 3:   # bottom halos: first pooled row of block r+1
            p1i[g:g + 32, 33, 1:129] = _pool2d(xb[32:64, 64 * r + 64:64 * r + 66], 2)[:, 0]
            p2i[g:g + 32, 17, 1:65] = _pool2d(xb[64:96, 64 * r + 64:64 * r + 68], 4)[:, 0]
            p3i[g:g + 32, 9, 1:33] = _pool2d(xb[96:128, 64 * r + 64:64 * r + 72], 8)[:, 0]

    return {
        "x": np.ascontiguousarray(xb.reshape(128, 4, 64, 256)),
        "x0": x0,
        "p1i": p1i, "p2i": p2i, "p3i": p3i,
    }


_PROGRAM_CACHE = {}


def build_program(act_func_name="Gelu"):
    key = act_func_name
    if key in _PROGRAM_CACHE:
        return _PROGRAM_CACHE[key]

    import concourse.bacc as bacc
    import concourse.tile as tile
    import concourse.mybir as mybir

    f32 = mybir.dt.float32
    bf16 = mybir.dt.bfloat16
    AOT = mybir.AluOpType
    act_func = getattr(mybir.ActivationFunctionType, act_func_name)

    nc = bacc.Bacc("TRN2", target_bir_lowering=False, debug=False)
    x_d = nc.dram_tensor("x", [CH, 4, 64, 256], bf16, kind="ExternalInput")
    x0_d = nc.dram_tensor("x0", [CH, NC_ * (CB + 2), W + 2], bf16,
                          kind="ExternalInput")
    wall_d = nc.dram_tensor("wall", [128, 12, 128], bf16, kind="ExternalInput")
    btot_d = nc.dram_tensor("btot", [128, 1], f32, kind="ExternalInput")
    wdwp_d = nc.dram_tensor("wdwp", [128, 3, 9], f32, kind="ExternalInput")
    p1i_d = nc.dram_tensor("p1i", [128, 34, 130], bf16, kind="ExternalInput")
    p2i_d = nc.dram_tensor("p2i", [128, 18, 66], bf16, kind="ExternalInput")
    p3i_d = nc.dram_tensor("p3i", [128, 10, 34], bf16, kind="ExternalInput")
    out_d = nc.dram_tensor("out", [CH, 4, 64, 256], f32, kind="ExternalOutput")

    mul_dt = bf16 if STORE_CAST else f32
    conv23_eng = None  # set inside

    with tile.TileContext(nc) as tc:
        with tc.tile_pool(name="persist", bufs=1) as pers, \
             tc.tile_pool(name="xband", bufs=3) as xpool, \
             tc.tile_pool(name="x0strip", bufs=2) as x0pool, \
             tc.tile_pool(name="ptmp", bufs=2) as ptmp, \
             tc.tile_pool(name="convb", bufs=2) as cpool, \
             tc.tile_pool(name="psum", bufs=8, space="PSUM") as pspool, \
             tc.tile_pool(name="gout", bufs=2) as gpool, \
             tc.tile_pool(name="mout", bufs=2) as mpool:

            conv23_eng = nc.gpsimd if CONV23_GPSIMD else nc.vector

            wall = pers.tile([128, 12, 128], bf16)
            nc.sync.dma_start(wall[:], wall_d[:])
            btot = pers.tile([128, 1], f32)
            nc.sync.dma_start(btot[:], btot_d[:])
            wdwp = pers.tile([128, 3, 9], f32)
            nc.sync.dma_start(wdwp[:], wdwp_d[:])

            p1pad = pers.tile([128, 34, 130], bf16)
            p2pad = pers.tile([128, 18, 66], bf16)
            p3pad = pers.tile([128, 10, 34], bf16)
            nc.sync.dma_start(p1pad[:], p1i_d[:])
            nc.sync.dma_start(p2pad[:], p2i_d[:])
            nc.sync.dma_start(p3pad[:], p3i_d[:])

            xbands = [None] * NC_
            x0s = [None] * NC_

            def load(c):
                xbands[c] = xpool.tile([128, 4, CB, 256], bf16, tag="xband",
                                       name=f"xband_{c}")
                nc.sync.dma_start(xbands[c][:], x_d[:, :, CB * c:CB * (c + 1), :])
                x0s[c] = x0pool.tile([128, CB + 2, 258], bf16, tag="x0",
                                     name=f"x0_{c}")
                nc.sync.dma_start(
                    x0s[c][:], x0_d[:, (CB + 2) * c:(CB + 2) * (c + 1), :])

            def pool(hb):
                # pool 8 image rows (half-band) of cband hb//2
                xband = xbands[hb // 2]
                ro = HB * (hb % 2)
                xs = xband[:, :, ro:ro + HB, :]
                b = hb   # strip-row indexing identical to 8-row bands
                v1 = ptmp.tile([128, 4, HB // 2, 256], bf16, tag="v1")
                nc.vector.tensor_tensor(
                    v1[:], xs[:, :, 0::2, :], xs[:, :, 1::2, :], AOT.max)
                p1t = ptmp.tile([128, 4, HB // 2, 128], bf16, tag="p1t")
                nc.vector.tensor_tensor(
                    p1t[:], v1[:, :, :, 0::2], v1[:, :, :, 1::2], AOT.max)
                v2 = ptmp.tile([128, 4, HB // 4, 128], bf16, tag="v2")
                nc.vector.tensor_tensor(
                    v2[:], p1t[:, :, 0::2, :], p1t[:, :, 1::2, :], AOT.max)
                p2t = ptmp.tile([128, 4, HB // 4, 64], bf16, tag="p2t")
                nc.vector.tensor_tensor(
                    p2t[:], v2[:, :, :, 0::2], v2[:, :, :, 1::2], AOT.max)
                v3 = ptmp.tile([128, 4, HB // 8, 64], bf16, tag="v3")
                nc.vector.tensor_tensor(
                    v3[:], p2t[:, :, 0::2, :], p2t[:, :, 1::2, :], AOT.max)
                p3t = ptmp.tile([128, 4, HB // 8, 32], bf16, tag="p3t")
                nc.vector.tensor_tensor(
                    p3t[:], v3[:, :, :, 0::2], v3[:, :, :, 1::2], AOT.max)
                for r in range(4):
                    g0 = r * 32
                    nc.sync.dma_start(
                        p1pad[g0:g0 + 32, 4 * b + 1:4 * b + 5, 1:129],
                        p1t[32:64, r])
                    nc.sync.dma_start(
                        p2pad[g0:g0 + 32, 2 * b + 1:2 * b + 3, 1:65],
                        p2t[64:96, r])
                    nc.sync.dma_start(
                        p3pad[g0:g0 + 32, b + 1:b + 2, 1:33],
                        p3t[96:128, r])

            def compute(c):
                # pooled convs for this cband's window (all 4 strips at once)
                conv1 = cpool.tile([128, 8, 128], bf16, tag="conv1")
                conv2 = cpool.tile([128, 4, 64], bf16, tag="conv2")
                conv3 = cpool.tile([128, 2, 32], bf16, tag="conv3")
                for j in range(9):
                    dy, dx = j // 3, j % 3
                    a1 = p1pad[:, 8 * c + dy:8 * c + dy + 8, dx:dx + 128]
                    a2 = p2pad[:, 4 * c + dy:4 * c + dy + 4, dx:dx + 64]
                    a3 = p3pad[:, 2 * c + dy:2 * c + dy + 2, dx:dx + 32]
                    if j == 0:
                        nc.vector.tensor_scalar_mul(conv1[:], a1, wdwp[:, 0, 0:1])
                        conv23_eng.tensor_scalar_mul(conv2[:], a2, wdwp[:, 1, 0:1])
                        conv23_eng.tensor_scalar_mul(conv3[:], a3, wdwp[:, 2, 0:1])
                    else:
                        nc.vector.scalar_tensor_tensor(
                            conv1[:], a1, wdwp[:, 0, j:j + 1], conv1[:],
                            AOT.mult, AOT.add)
                        conv23_eng.scalar_tensor_tensor(
                            conv2[:], a2, wdwp[:, 1, j:j + 1], conv2[:],
                            AOT.mult, AOT.add)
                        conv23_eng.scalar_tensor_tensor(
                            conv3[:], a3, wdwp[:, 2, j:j + 1], conv3[:],
                            AOT.mult, AOT.add)

                xband, x0 = xbands[c], x0s[c]
                for ip in range(CB // 4):          # chunk pairs (4 rows)
                    mt = mpool.tile([128, 4, 4, 256], mul_dt, tag="mchunk")
                    gt = gpool.tile([128, 4, 4, 256], bf16, tag="gchunk")
                    for ih in range(2):
                        i = 2 * ip + ih            # chunk (2 rows)
                        pss = [pspool.tile([128, 2, 256], f32, tag="pschunk",
                                           name=f"ps_{c}_{i}_{r}")
                               for r in range(4)]
                        # x0 taps first (only need the x0 DMA), conv-dependent
                        # slots last so convs stay off the chunk critical path
                        for t in range(12):
                            for r in range(4):
                                g0 = 32 * r
                                if t >= 9:
                                    lhsT = wall[g0:g0 + 32, t, :]
                                    if t == 9:
                                        rhs = conv1[g0:g0 + 32, i, :] \
                                            .unsqueeze(1).unsqueeze(3) \
                                            .broadcast_to([32, 2, 128, 2])
                                    elif t == 10:
                                        rhs = conv2[g0:g0 + 32, i // 2, :] \
                                            .unsqueeze(1).unsqueeze(3) \
                                            .broadcast_to([32, 2, 64, 4])
                                    else:
                                        rhs = conv3[g0:g0 + 32, i // 4, :] \
                                            .unsqueeze(1).unsqueeze(3) \
                                            .broadcast_to([32, 2, 32, 8])
                                else:
                                    j = t
                                    dy, dx = j // 3, j % 3
                                    lhsT = wall[g0:g0 + 32, j, :]
                                    rhs = x0[g0:g0 + 32,
                                             2 * i + dy:2 * i + dy + 2,
                                             dx:dx + 256]
                                nc.tensor.matmul(
                                    pss[r][:], lhsT, rhs,
                                    start=(t == 0), stop=(t == 11),
                                    tile_position=(g0, 0))
                        for r in range(4):
                            nc.scalar.activation(
                                gt[:, r, 2 * ih:2 * ih + 2, :], pss[r][:],
                                act_func, bias=btot[:, 0:1])
                    nc.vector.tensor_mul(
                        mt[:], gt[:], xband[:, :, 4 * ip:4 * ip + 4, :])
                    h = CB * c + 4 * ip
                    if STORE_CAST:
                        nc.gpsimd.dma_start(out_d[:, :, h:h + 4, :], mt[:])
                    else:
                        nc.sync.dma_start(out_d[:, :, h:h + 4, :], mt[:])

            # software pipeline: pooling runs one half-band ahead of compute
            load(0)
            load(1)
            pool(0)
            pool(1)
            for c in range(NC_):
                if 2 * c + 2 < 2 * NC_:
                    pool(2 * c + 2)
                compute(c)
                if 2 * c + 3 < 2 * NC_:
                    pool(2 * c + 3)
                if c + 2 < NC_:
                    load(c + 2)

    nc.compile()
    _PROGRAM_CACHE[key] = nc
    return nc


def make_in_maps(x, w_dw, b_dw, w_f1, b_f1, w_f2, b_f2, w_f3, b_f3):
    W_all, b_tot, wdwp = fold_weights(
        np.asarray(w_dw), np.asarray(b_dw), np.asarray(w_f1), np.asarray(b_f1),
        np.asarray(w_f2), np.asarray(b_f2), np.asarray(w_f3), np.asarray(b_f3))
    x = np.asarray(x)
    in_maps = []
    for i in range(x.shape[0]):
        m = prep_sample(np.ascontiguousarray(x[i], dtype=np.float32))
        m.update({"wall": W_all, "btot": b_tot, "wdwp": wdwp})
        in_maps.append(m)
    return in_maps


def kernel(x, w_dw, b_dw, w_f1, b_f1, w_f2, b_f2, w_f3, b_f3):
    from concourse.bass_utils import run_bass_kernel_spmd

    x = np.asarray(x)
    B = x.shape[0]
    in_maps = make_in_maps(x, w_dw, b_dw, w_f1, b_f1, w_f2, b_f2, w_f3, b_f3)
    nc = build_program("Gelu")
    res = run_bass_kernel_spmd(nc, in_maps, list(range(B)))
    out = np.stack([res.results[i]["out"].reshape(CH, H, W)
                    for i in range(B)], axis=0)
    return out.astype(np.float32)


# revision 13
# speedup vs baseline: 1.0946x; 1.0946x over previous
"""Trainium2 Bass kernel for nn_CheapChannelV1 (dense_cnn).

Strategy (per core, pure data-parallel over batch — one sample per core):
  - The three channel-shuffle + 1x1-conv stages are linear, so they fold on the
    host into ONE 128x128 matrix M and bias b_tot:  res3 = M @ s + b_tot, where
    s = [s0;s1;s2;s3] are the four depthwise-conv branch outputs.
  - All matmul operands are bf16 (fp32 PSUM accumulation): fp32 matmuls run at
    4 cycles/column on the PE vs 1 for bf16.  x is cast to bf16 on the host,
    which also halves the HBM read traffic.
  - Level-0 depthwise conv (full res) folds INTO the matmul: 9 tap matmuls
    (K=32) reading shifted views of a host-prepadded x0 strip (channels 0-31
    replicated across the four 32-partition groups, one group per row-block).
  - Levels 1-3: hierarchical 2x2 max-pool on DVE in 8-row half-bands
    (vertical-first so half the ops hit the 16-bit 2x mode); 3x3 depthwise
    conv over 16-row bands (level 1 on DVE, levels 2+3 on GPSIMD);
    nearest-upsample folds into broadcast (step-0) rhs APs of the matmuls.
  - Pooling runs one half-band ahead of compute so the conv's +1-row halo
    dependency never serializes the band pipeline.  Block-boundary halo rows
    are seeded from a tiny host-computed init tensor.
  - 12 accumulating K=32 matmuls per 512-px chunk, spread across the four PE
    row groups via tile_position for 4x concurrency.
  - Epilogue: exact Gelu on ACT (bias folded in, bf16 out), multiply-by-x on
    DVE in bf16 (2x mode), store via SWDGE cast-DMA (bf16 SBUF -> fp32 HBM).
"""

import numpy as np
import ml_dtypes

BF16 = ml_dtypes.bfloat16

H = W = 256
CH = 128
NC_ = 4       # compute bands ("cbands") of 16 rows per row-block
CB = 16       # rows per cband
HB = 8        # half-band rows (pooling granularity)

STORE_CAST = True     # bf16 mul output + SWDGE cast store (else fp32 + sync)


def _shuf_cols(A, groups=8):
    # Returns A' with A' @ s == A @ channel_shuffle(s)
    Cin = A.shape[1]
    idx = np.arange(Cin)
    perm = (idx % groups) * (Cin // groups) + idx // groups
    Ap = np.zeros_like(A)
    Ap[:, perm] = A
    return Ap


def fold_weights(w_dw, b_dw, w_f1, b_f1, w_f2, b_f2, w_f3, b_f3):
    f8 = np.float64
    A1 = _shuf_cols(w_f1.astype(f8))
    A2 = _shuf_cols(w_f2.astype(f8))
    A3 = _shuf_cols(w_f3.astype(f8))
    A2a, A2b = A2[:, :64], A2[:, 64:]
    A3a, A3b = A3[:, :96], A3[:, 96:]
    M = np.zeros((128, 128), f8)
    M[:, 0:64] = A3a @ A2a @ A1
    M[:, 64:96] = A3a @ A2b
    M[:, 96:128] = A3b
    b_tot = A3a @ (A2a @ b_f1.astype(f8) + b_f2.astype(f8)) + b_f3.astype(f8)
    for g in range(4):
        b_tot = b_tot + M[:, 32 * g:32 * g + 32] @ b_dw[g].astype(f8)

    # W_all[p, t, o]: lhsT matrices, identical content per 32-partition group.
    W_all = np.zeros((128, 12, 128), np.float32)
    M0T = M[:, 0:32].T          # [32(c), 128(o)]
    w0 = w_dw[0].reshape(32, 9).astype(f8)
    for gp in range(4):
        rows = slice(32 * gp, 32 * gp + 32)
        for j in range(9):
            W_all[rows, j, :] = (M0T * w0[:, j:j + 1]).astype(np.float32)
        W_all[rows, 9, :] = M[:, 32:64].T.astype(np.float32)
        W_all[rows, 10, :] = M[:, 64:96].T.astype(np.float32)
        W_all[rows, 11, :] = M[:, 96:128].T.astype(np.float32)

    # wdiag[32r+c, j, g-1, c'] = diag depthwise-tap lhsT for PE conv matmuls
    wdiag = np.zeros((128, 9, 3, 32), np.float32)
    for g in (1, 2, 3):
        wg = w_dw[g].reshape(32, 9).astype(np.float32)   # [c, j]
        for r in range(4):
            for c in range(32):
                wdiag[32 * r + c, :, g - 1, c] = wg[c, :]

    return (np.ascontiguousarray(W_all.astype(BF16)),
            b_tot.astype(np.float32).reshape(128, 1),
            np.ascontiguousarray(wdiag.astype(BF16)))


def _pool2d(a, k):
    # a: [C, R, W] -> max-pooled [C, R//k, W//k]
    C, R, Ww = a.shape
    return a.reshape(C, R // k, k, Ww // k, k).max(axis=(2, 4))


def prep_sample(x):
    """Host-side layout/dtype prep for one sample x [128, 256, 256] fp32."""
    xb = x.astype(BF16)

    # x0 strip: channels 0-31 replicated to the 4 row-block partition groups,
    # pre-padded; cband c rows are image rows 16c-1 .. 16c+17 (block-local),
    # cols padded by 1 on each side.
    xp = np.zeros((32, H + 2, W + 2), BF16)
    xp[:, 1:H + 1, 1:W + 1] = xb[:32]
    rows = (np.arange(4)[:, None, None] * 64
            + np.arange(NC_)[None, :, None] * CB
            + np.arange(CB + 2)[None, None, :])       # [4, 4, 18] (+1 pad -1)
    x0 = xp[:, rows.reshape(-1), :]                    # [32, 288, 258]
    x0 = np.ascontiguousarray(
        x0.reshape(32, 4, NC_ * (CB + 2), W + 2).transpose(1, 0, 2, 3)
        .reshape(128, NC_ * (CB + 2), W + 2))

    # Pool-strip pad inits: zeros + block-boundary halo rows.
    p1i = np.zeros((128, 34, 130), BF16)
    p2i = np.zeros((128, 18, 66), BF16)
    p3i = np.zeros((128, 10, 34), BF16)
    for r in range(4):
        g = 32 * r
        if r > 0:   # top halos: last pooled row of block r-1
            p1i[g:g + 32, 0, 1:129] = _pool2d(xb[32:64, 64 * r - 2:64 * r], 2)[:, 0]
            p2i[g:g + 32, 0, 1:65] = _pool2d(xb[64:96, 64 * r - 4:64 * r], 4)[:, 0]
            p3i[g:g + 32, 0, 1:33] = _pool2d(xb[96:128, 64 * r - 8:64 * r], 8)[:, 0]
        if r < 3:   # bottom halos: first pooled row of block r+1
            p1i[g:g + 32, 33, 1:129] = _pool2d(xb[32:64, 64 * r + 64:64 * r + 66], 2)[:, 0]
            p2i[g:g + 32, 17, 1:65] = _pool2d(xb[64:96, 64 * r + 64:64 * r + 68], 4)[:, 0]
            p3i[g:g + 32, 9, 1:33] = _pool2d(xb[96:128, 64 * r + 64:64 * r + 72], 8)[:, 0]

    return {
        "x": np.ascontiguousarray(xb.reshape(128, 4, 64, 256)),
        "x0": x0,
        "p1i": p1i, "p2i": p2i, "p3i": p3i,
    }


_PROGRAM_CACHE = {}


def build_program(act_func_name="Gelu"):
    key = act_func_name
    if key in _PROGRAM_CACHE:
        return _PROGRAM_CACHE[key]

    import concourse.bacc as bacc
    import concourse.tile as tile
    import concourse.mybir as mybir

    f32 = mybir.dt.float32
    bf16 = mybir.dt.bfloat16
    AOT = mybir.AluOpType
    act_func = getattr(mybir.ActivationFunctionType, act_func_name)

    nc = bacc.Bacc("TRN2", target_bir_lowering=False, debug=False)
    x_d = nc.dram_tensor("x", [CH, 4, 64, 256], bf16, kind="ExternalInput")
    x0_d = nc.dram_tensor("x0", [CH, NC_ * (CB + 2), W + 2], bf16,
                          kind="ExternalInput")
    wall_d = nc.dram_tensor("wall", [128, 12, 128], bf16, kind="ExternalInput")
    btot_d = nc.dram_tensor("btot", [128, 1], f32, kind="ExternalInput")
    wdiag_d = nc.dram_tensor("wdiag", [128, 9, 3, 32], bf16,
                             kind="ExternalInput")
    p1i_d = nc.dram_tensor("p1i", [128, 34, 130], bf16, kind="ExternalInput")
    p2i_d = nc.dram_tensor("p2i", [128, 18, 66], bf16, kind="ExternalInput")
    p3i_d = nc.dram_tensor("p3i", [128, 10, 34], bf16, kind="ExternalInput")
    out_d = nc.dram_tensor("out", [CH, 4, 64, 256], f32, kind="ExternalOutput")

    mul_dt = bf16 if STORE_CAST else f32

    with tile.TileContext(nc) as tc:
        with tc.tile_pool(name="persist", bufs=1) as pers, \
             tc.tile_pool(name="xband", bufs=3) as xpool, \
             tc.tile_pool(name="x0strip", bufs=2) as x0pool, \
             tc.tile_pool(name="ptmp", bufs=2) as ptmp, \
             tc.tile_pool(name="convb", bufs=2) as cpool, \
             tc.tile_pool(name="psum", bufs=8, space="PSUM") as pspool, \
             tc.tile_pool(name="gout", bufs=2) as gpool, \
             tc.tile_pool(name="mout", bufs=2) as mpool:

            wall = pers.tile([128, 12, 128], bf16)
            nc.sync.dma_start(wall[:], wall_d[:])
            btot = pers.tile([128, 1], f32)
            nc.sync.dma_start(btot[:], btot_d[:])
            wdiag = pers.tile([128, 9, 3, 32], bf16)
            nc.sync.dma_start(wdiag[:], wdiag_d[:])

            p1pad = pers.tile([128, 34, 130], bf16)
            p2pad = pers.tile([128, 18, 66], bf16)
            p3pad = pers.tile([128, 10, 34], bf16)
            nc.sync.dma_start(p1pad[:], p1i_d[:])
            nc.sync.dma_start(p2pad[:], p2i_d[:])
            nc.sync.dma_start(p3pad[:], p3i_d[:])

            xbands = [None] * NC_
            x0s = [None] * NC_

            def load(c):
                xbands[c] = xpool.tile([128, 4, CB, 256], bf16, tag="xband",
                                       name=f"xband_{c}")
                nc.sync.dma_start(xbands[c][:], x_d[:, :, CB * c:CB * (c + 1), :])
                x0s[c] = x0pool.tile([128, CB + 2, 258], bf16, tag="x0",
                                     name=f"x0_{c}")
                nc.sync.dma_start(
                    x0s[c][:], x0_d[:, (CB + 2) * c:(CB + 2) * (c + 1), :])

            def pool(hb):
                # pool 8 image rows (half-band) of cband hb//2
                xband = xbands[hb // 2]
                ro = HB * (hb % 2)
                xs = xband[:, :, ro:ro + HB, :]
                b = hb   # strip-row indexing identical to 8-row bands
                v1 = ptmp.tile([128, 4, HB // 2, 256], bf16, tag="v1")
                nc.vector.tensor_tensor(
                    v1[:], xs[:, :, 0::2, :], xs[:, :, 1::2, :], AOT.max)
                p1t = ptmp.tile([128, 4, HB // 2, 128], bf16, tag="p1t")
                nc.vector.tensor_tensor(
                    p1t[:], v1[:, :, :, 0::2], v1[:, :, :, 1::2], AOT.max)
                v2 = ptmp.tile([128, 4, HB // 4, 128], bf16, tag="v2")
                nc.vector.tensor_tensor(
                    v2[:], p1t[:, :, 0::2, :], p1t[:, :, 1::2, :], AOT.max)
                p2t = ptmp.tile([128, 4, HB // 4, 64], bf16, tag="p2t")
                nc.vector.tensor_tensor(
                    p2t[:], v2[:, :, :, 0::2], v2[:, :, :, 1::2], AOT.max)
                v3 = ptmp.tile([128, 4, HB // 8, 64], bf16, tag="v3")
                nc.vector.tensor_tensor(
                    v3[:], p2t[:, :, 0::2, :], p2t[:, :, 1::2, :], AOT.max)
                p3t = ptmp.tile([128, 4, HB // 8, 32], bf16, tag="p3t")
                nc.vector.tensor_tensor(
                    p3t[:], v3[:, :, :, 0::2], v3[:, :, :, 1::2], AOT.max)
                for r in range(4):
                    g0 = r * 32
                    nc.sync.dma_start(
                        p1pad[g0:g0 + 32, 4 * b + 1:4 * b + 5, 1:129],
                        p1t[32:64, r])
                    nc.sync.dma_start(
                        p2pad[g0:g0 + 32, 2 * b + 1:2 * b + 3, 1:65],
                        p2t[64:96, r])
                    nc.sync.dma_start(
                        p3pad[g0:g0 + 32, b + 1:b + 2, 1:33],
                        p3t[96:128, r])

            def compute(c):
                # pooled convs for this cband's window: PE diagonal-lhsT
                # matmuls, 9 accumulating taps into PSUM, then ACT copy to
                # SBUF bf16.  tile_position=(g0, g0): contraction rows AND
                # output partitions follow the strip's partition group.
                conv1 = cpool.tile([128, 8, 128], bf16, tag="conv1")
                conv2 = cpool.tile([128, 4, 64], bf16, tag="conv2")
                conv3 = cpool.tile([128, 2, 32], bf16, tag="conv3")
                cps1 = [pspool.tile([128, 4, 128], f32, tag="pschunk",
                                    name=f"cps1_{c}_{h}") for h in range(2)]
                cps2 = pspool.tile([128, 4, 64], f32, tag="pschunk",
                                   name=f"cps2_{c}")
                cps3 = pspool.tile([128, 2, 32], f32, tag="pschunk",
                                   name=f"cps3_{c}")
                for j in range(9):
                    dy, dx = j // 3, j % 3
                    for r in range(4):
                        g0 = 32 * r
                        for h in range(2):
                            nc.tensor.matmul(
                                cps1[h][g0:g0 + 32],
                                wdiag[g0:g0 + 32, j, 0, :],
                                p1pad[g0:g0 + 32,
                                      8 * c + 4 * h + dy:8 * c + 4 * h + dy + 4,
                                      dx:dx + 128],
                                start=(j == 0), stop=(j == 8),
                                tile_position=(g0, g0))
                        nc.tensor.matmul(
                            cps2[g0:g0 + 32],
                            wdiag[g0:g0 + 32, j, 1, :],
                            p2pad[g0:g0 + 32, 4 * c + dy:4 * c + dy + 4,
                                  dx:dx + 64],
                            start=(j == 0), stop=(j == 8),
                            tile_position=(g0, g0))
                        nc.tensor.matmul(
                            cps3[g0:g0 + 32],
                            wdiag[g0:g0 + 32, j, 2, :],
                            p3pad[g0:g0 + 32, 2 * c + dy:2 * c + dy + 2,
                                  dx:dx + 32],
                            start=(j == 0), stop=(j == 8),
                            tile_position=(g0, g0))
                copy_f = mybir.ActivationFunctionType.Copy
                nc.scalar.activation(conv1[:, 0:4], cps1[0][:], copy_f)
                nc.scalar.activation(conv1[:, 4:8], cps1[1][:], copy_f)
                nc.scalar.activation(conv2[:], cps2[:], copy_f)
                nc.scalar.activation(conv3[:], cps3[:], copy_f)

                xband, x0 = xbands[c], x0s[c]
                for ip in range(CB // 4):          # chunk pairs (4 rows)
                    mt = mpool.tile([128, 4, 4, 256], mul_dt, tag="mchunk")
                    gt = gpool.tile([128, 4, 4, 256], bf16, tag="gchunk")
                    for ih in range(2):
                        i = 2 * ip + ih            # chunk (2 rows)
                        pss = [pspool.tile([128, 2, 256], f32, tag="pschunk",
                                           name=f"ps_{c}_{i}_{r}")
                               for r in range(4)]
                        # x0 taps first (only need the x0 DMA), conv-dependent
                        # slots last so convs stay off the chunk critical path
                        for t in range(12):
                            for r in range(4):
                                g0 = 32 * r
                                if t >= 9:
                                    lhsT = wall[g0:g0 + 32, t, :]
                                    if t == 9:
                                        rhs = conv1[g0:g0 + 32, i, :] \
                                            .unsqueeze(1).unsqueeze(3) \
                                            .broadcast_to([32, 2, 128, 2])
                                    elif t == 10:
                                        rhs = conv2[g0:g0 + 32, i // 2, :] \
                                            .unsqueeze(1).unsqueeze(3) \
                                            .broadcast_to([32, 2, 64, 4])
                                    else:
                                        rhs = conv3[g0:g0 + 32, i // 4, :] \
                                            .unsqueeze(1).unsqueeze(3) \
                                            .broadcast_to([32, 2, 32, 8])
                                else:
                                    j = t
                                    dy, dx = j // 3, j % 3
                                    lhsT = wall[g0:g0 + 32, j, :]
                                    rhs = x0[g0:g0 + 32,
                                             2 * i + dy:2 * i + dy + 2,
                                             dx:dx + 256]
                                nc.tensor.matmul(
                                    pss[r][:], lhsT, rhs,
                                    start=(t == 0), stop=(t == 11),
                                    tile_position=(g0, 0))
                        for r in range(4):
                            nc.scalar.activation(
                                gt[:, r, 2 * ih:2 * ih + 2, :], pss[r][:],
                                act_func, bias=btot[:, 0:1])
                    nc.vector.tensor_mul(
                        mt[:], gt[:], xband[:, :, 4 * ip:4 * ip + 4, :])
                    h = CB * c + 4 * ip
                    if STORE_CAST:
                        nc.gpsimd.dma_start(out_d[:, :, h:h + 4, :], mt[:])
                    else:
                        nc.sync.dma_start(out_d[:, :, h:h + 4, :], mt[:])

            # software pipeline: pooling runs one half-band ahead of compute
            load(0)
            load(1)
            pool(0)
            pool(1)
            for c in range(NC_):
                if 2 * c + 2 < 2 * NC_:
                    pool(2 * c + 2)
                compute(c)
                if 2 * c + 3 < 2 * NC_:
                    pool(2 * c + 3)
                if c + 2 < NC_:
                    load(c + 2)

    nc.compile()
    _PROGRAM_CACHE[key] = nc
    return nc


def make_in_maps(x, w_dw, b_dw, w_f1, b_f1, w_f2, b_f2, w_f3, b_f3):
    W_all, b_tot, wdiag = fold_weights(
        np.asarray(w_dw), np.asarray(b_dw), np.asarray(w_f1), np.asarray(b_f1),
        np.asarray(w_f2), np.asarray(b_f2), np.asarray(w_f3), np.asarray(b_f3))
    x = np.asarray(x)
    in_maps = []
    for i in range(x.shape[0]):
        m = prep_sample(np.ascontiguousarray(x[i], dtype=np.float32))
        m.update({"wall": W_all, "btot": b_tot, "wdiag": wdiag})
        in_maps.append(m)
    return in_maps


def kernel(x, w_dw, b_dw, w_f1, b_f1, w_f2, b_f2, w_f3, b_f3):
    from concourse.bass_utils import run_bass_kernel_spmd

    x = np.asarray(x)
    B = x.shape[0]
    in_maps = make_in_maps(x, w_dw, b_dw, w_f1, b_f1, w_f2, b_f2, w_f3, b_f3)
    nc = build_program("Gelu")
    res = run_bass_kernel_spmd(nc, in_maps, list(range(B)))
    out = np.stack([res.results[i]["out"].reshape(CH, H, W)
                    for i in range(B)], axis=0)
    return out.astype(np.float32)
